# revision 60
# baseline (speedup 1.0000x reference)
"""Sliding-window attention kernel for 8 TRN2 NeuronCores.

Sharding: core c owns heads {2c, 2c+1} for BOTH batches (tensor parallel
over the 16 heads).  After attention, an all-to-all redistributes the
per-head outputs so core c owns output rows (batch c//4, t-chunk c%4),
where it applies the full Wo projection.

Precision plan (rel-err budget 2e-2):
  - Q/K projections: fp8e4m3 x and 64*W (scale dodges e4m3 subnormals),
    DoubleRow perf mode; the 64*64 factor is folded into the exp scale.
  - scores: fp8 DoubleRow via a [32, 2, T] repack of Q'/K'.
  - V / AV / Wo: bf16 (V-path and Wo-path errors hit the output at full
    relative strength, so they stay 16-bit).
  - output: bf16 on device, upcast to f32 on host.

Q/K head-dim permutation: partition j of a 64-row head block holds dim
  dl(j) = 16*(j//32) + (j%16) + 32*(j%32 >= 16)
so each RoPE pair (d, d+32) sits 16 partitions apart inside one
32-partition block, letting DVE stream_shuffle do the rotate-half swap
(no DMA).  Scores are dot products over d, so any permutation shared by
Q and K is transparent; cos/sin tables are built in permuted order.

Per-core pipeline (t-chunks of 512 tokens, batches interleaved so
attention can start after ~1/4 of the projections):
  chunk: load x8/xb -> QK proj (PE fp8-DR) -> DVE copy bf16 -> RoPE
         (shuffle + 2 mul + add-to-fp8) -> repack DMA to [32, 2, T];
         V^T proj (PE bf16) -> PE transpose -> V natural + ones (v_aug)
  attention per (h2, kt, b): banded S^T (PE fp8-DR, 32x2 layout) ->
         exp (ACT) -> boundary masks (DVE left / Pool right) -> E bf16;
         AV per (b, qc) (PE bf16), denominator = ones row
  two AllToAlls (per h2); recv work for h2=0 is emitted AFTER h2=1's
  attention so the in-order engine queues never couple h2=1 behind the
  first collective.  Wo half per h2 (h2=0 overlaps collective #2).
"""
import numpy as np
import ml_dtypes

import concourse.bass as bass
import concourse.bacc as bacc
import concourse.mybir as mybir
import concourse.tile as tile

F32 = mybir.dt.float32
BF16 = mybir.dt.bfloat16
FP8 = mybir.dt.float8e4
AF = mybir.ActivationFunctionType
ALU = mybir.AluOpType
DR = mybir.MatmulPerfMode.DoubleRow

B, T, D = 2, 2048, 1024
H, DH = 16, 64
WIN = T // 4              # 512
N_CORES = 8
HPC = H // N_CORES        # heads per core = 2
TC = T // 4               # output t-chunk per core = 512
KT = T // 128             # k-tiles per (head,batch) = 16
SCALE = 1.0 / np.sqrt(DH)

NKT = 128                 # k-tile rows
MAXW = 1152               # max window width per k-tile

# RoPE-friendly permutation: partition j (of 64) -> head dim dl(j)
_j = np.arange(64)
DPERM = 16 * (_j // 32) + (_j % 16) + 32 * ((_j % 32) >= 16)
SHUF_MASK = list(range(16, 32)) + list(range(0, 16))


def window(k0):
    """q-window [ws, we) for k-tile starting at k0."""
    return max(k0 - WIN, 0), min(k0 + NKT + WIN, T)


EOFF = []
_off = 0
for _kt in range(KT):
    _ws, _we = window(_kt * 128)
    EOFF.append(_off)
    _off += _we - _ws
ETOT = _off


def host_inputs(x, Wq, Wk, Wv, Wo, core):
    """Build the per-core input map (host-side shard + constant tables)."""
    bf = ml_dtypes.bfloat16
    f8 = ml_dtypes.float8_e4m3fn
    cols = slice(core * HPC * DH, (core + 1) * HPC * DH)
    xf = x.reshape(B * T, D)                              # [4096, 1024]
    xt = np.ascontiguousarray(xf.T)                       # [1024, 4096]
    # bf16 x [128, 8 cb, 4096]
    xbf = xt.reshape(8, 128, B * T).transpose(1, 0, 2)
    # permuted output-column order for Q/K: partition p -> col pcol[p]
    pcol = np.concatenate([DPERM, 64 + DPERM])            # [128]
    def wqk(W):
        w = W[:, cols][:, pcol].reshape(8, 128, HPC * DH)
        return np.ascontiguousarray(w.transpose(1, 0, 2).astype(bf))
    wvb = Wv[:, cols].reshape(8, 128, HPC * DH).transpose(1, 0, 2)
    wob = Wo.reshape(8, 128, D).transpose(1, 0, 2)
    t = np.arange(T, dtype=np.float64)
    inv = 1.0 / (10000.0 ** (np.arange(0, DH, 2, dtype=np.float64) / DH))
    f = (t[:, None] * inv[None, :]).astype(np.float32)   # [T, 32]
    cos1 = np.cos(f).astype(np.float32)                  # [T, 32]
    sin1 = np.sin(f).astype(np.float32)
    # permuted ^T tables [128, T]
    dl = pcol % 64                                       # head-dim per row
    cos_t = cos1.T[dl % 32]                              # [128, T]
    sin_t = sin1.T[dl % 32]
    sgn = np.where(dl < 32, -1.0, 1.0).astype(np.float32)[:, None]
    sin_s = sin_t * sgn                                  # signed sin for swap
    kr = np.arange(128)[:, None]
    qc = np.arange(128)[None, :]
    tri_l = (kr <= qc).astype(np.float32)                # valid, left boundary
    tri_r = (qc <= kr).astype(np.float32)                # valid, right boundary
    return {
        "xbf": np.ascontiguousarray(xbf.astype(bf)),
        "wq8": wqk(Wq),
        "wk8": wqk(Wk),
        "wvb": np.ascontiguousarray(wvb.astype(bf)),
        "wob": np.ascontiguousarray(wob.astype(bf)),
        "cosb": cos_t.astype(bf),
        "sinb": sin_s.astype(bf),
        "tril": tri_l,
        "trir": tri_r,
        "idn": np.eye(128, dtype=bf),
    }


def host_assemble(results):
    """Concatenate the 8 per-core [512, 1024] chunks into [B, T, D]."""
    out = np.empty((B, T, D), np.float32)
    for c in range(N_CORES):
        out[c // 4, (c % 4) * TC:(c % 4 + 1) * TC, :] = \
            results[c]["out"].astype(np.float32)
    return out


def build(nc, replicate=1, debug=False):
    xbf_d = nc.dram_tensor("xbf", [128, 8, B * T], BF16, kind="ExternalInput").ap()
    wq_d = nc.dram_tensor("wq8", [128, 8, HPC * DH], BF16, kind="ExternalInput").ap()
    wk_d = nc.dram_tensor("wk8", [128, 8, HPC * DH], BF16, kind="ExternalInput").ap()
    wv_d = nc.dram_tensor("wvb", [128, 8, HPC * DH], BF16, kind="ExternalInput").ap()
    wo_d = nc.dram_tensor("wob", [128, 8, D], BF16, kind="ExternalInput").ap()
    cos_d = nc.dram_tensor("cosb", [128, T], BF16, kind="ExternalInput").ap()
    sin_d = nc.dram_tensor("sinb", [128, T], BF16, kind="ExternalInput").ap()
    tl_d = nc.dram_tensor("tril", [128, 128], F32, kind="ExternalInput").ap()
    tr_d = nc.dram_tensor("trir", [128, 128], F32, kind="ExternalInput").ap()
    idn_d = nc.dram_tensor("idn", [128, 128], BF16, kind="ExternalInput").ap()
    out_d = nc.dram_tensor("out", [TC, D], BF16, kind="ExternalOutput").ap()
    dbg = {}
    if debug:
        for name, shape, dt_ in [
            ("dbg_q", [128, B, T], BF16),
            ("dbg_k", [128, B, T], BF16),
            ("dbg_vaug", [128, B * KT, 130], BF16),
            ("dbg_e0", [128, ETOT], BF16),
            ("dbg_e1", [128, ETOT], BF16),
            ("dbg_a", [65, T], BF16),
        ]:
            dbg[name] = nc.dram_tensor(name, shape, dt_,
                                       kind="ExternalOutput").ap()
    with tile.TileContext(nc) as tc:
        for _ in range(replicate):
            _build_once(nc, tc, xbf_d, wq_d, wk_d, wv_d, wo_d,
                        cos_d, sin_d, tl_d, tr_d, idn_d, out_d, dbg)
    nc.compile()
    return nc


def _build_once(nc, tc, xbf_d, wq_d, wk_d, wv_d, wo_d, cos_d, sin_d,
                tl_d, tr_d, idn_d, out_d, dbg={}):
    with tc.tile_pool(name="const", bufs=1) as constp, \
         tc.tile_pool(name="xs", bufs=2) as xsp, \
         tc.tile_pool(name="qk", bufs=1) as qkp, \
         tc.tile_pool(name="rope", bufs=2) as ropep, \
         tc.tile_pool(name="ebuf", bufs=2) as ep, \
         tc.tile_pool(name="abuf", bufs=1) as ap_, \
         tc.tile_pool(name="fin", bufs=1) as finp, \
         tc.tile_pool(name="ps", bufs=2, space="PSUM") as ps, \
         tc.tile_pool(name="dram", bufs=1, space="DRAM") as dr:

        # ---------------- constants / weights ----------------
        wq_sb = constp.tile([128, 8, HPC * DH], BF16, tag="wq")
        wk_sb = constp.tile([128, 8, HPC * DH], BF16, tag="wk")
        wv_sb = constp.tile([128, 8, HPC * DH], BF16, tag="wv")
        wo_sb = constp.tile([128, 8, D], BF16, tag="wo")
        cos_sb = constp.tile([128, T], BF16, tag="cos")
        sin_sb = constp.tile([128, T], BF16, tag="sin")
        tl_sb = constp.tile([128, 128], F32, tag="tl")
        tr_sb = constp.tile([128, 128], F32, tag="tr")
        idn_sb = constp.tile([128, 128], BF16, tag="idn")
        nc.sync.dma_start(wq_sb[:], wq_d[:])
        nc.sync.dma_start(wk_sb[:], wk_d[:])

        # ---------------- projections + RoPE (8 chunks, b interleaved) ----
        v_aug = ap_.tile([128, B * KT, 130], BF16, tag="vaug")
        one_view = v_aug[:].rearrange("p k (h e) -> p k h e", e=65)[:, :, :, 64]
        nc.vector.memset(one_view, 1.0)
        # RoPE'd Q'/K' bf16
        qrope = qkp.tile([128, B, T], BF16, tag="qr")
        krope = qkp.tile([128, B, T], BF16, tag="kr")
        xbs = {}

        def do_chunk_qk(b, tq):
            g0 = b * T + tq * 512
            tt0 = tq * 512
            xb = xsp.tile([128, 8, 512], BF16, tag="xb")
            nc.sync.dma_start(xb[:], xbf_d[:, :, g0:g0 + 512])
            for wsb, dstr in ((wq_sb, qrope), (wk_sb, krope)):
                pt = ps.tile([128, 512], F32, tag="pp")
                for cb in range(8):
                    nc.tensor.matmul(pt[:], wsb[:, cb, :], xb[:, cb, :],
                                     start=(cb == 0), stop=(cb == 7))
                qc_ch = ropep.tile([128, 512], BF16, tag="qc")
                nc.scalar.activation(qc_ch[:], pt[:], AF.Copy)
                # RoPE on the chunk (pairs sit 16 partitions apart).
                # f32 tables/intermediates force 1x-mode DVE ops: the
                # packed 2x/4x modes half-write at some SBUF alignments.
                sw = ropep.tile([128, 512], BF16, tag="sw")
                qcf = ropep.tile([128, 512], F32, tag="qcf", bufs=1)
                swf = ropep.tile([128, 512], F32, tag="swf", bufs=1)
                nc.vector.stream_shuffle(sw[:], qc_ch[:], SHUF_MASK)
                nc.vector.tensor_mul(qcf[:], qc_ch[:],
                                     cos_sb[:, tt0:tt0 + 512])
                nc.vector.tensor_mul(swf[:], sw[:],
                                     sin_sb[:, tt0:tt0 + 512])
                nc.vector.tensor_add(dstr[:, b, tt0:tt0 + 512],
                                     qcf[:], swf[:])

        def do_chunk_v(b, tq):
            g0 = b * T + tq * 512
            xb = xsp.tile([128, 8, 512], BF16, tag="xb")
            nc.sync.dma_start(xb[:], xbf_d[:, :, g0:g0 + 512])
            pt = ps.tile([128, 512], F32, tag="pp")
            for cb in range(8):
                nc.tensor.matmul(pt[:], wv_sb[:, cb, :], xb[:, cb, :],
                                 start=(cb == 0), stop=(cb == 7))
            vt_sb = ropep.tile([128, 512], BF16, tag="vt")
            nc.scalar.activation(vt_sb[:], pt[:], AF.Copy)
            for ktl in range(4):
                gkt = b * KT + tq * 4 + ktl
                tp = ps.tile([128, 128], BF16, tag="pp",
                             padded_shape=[128, 1024])
                nc.tensor.transpose(tp[:],
                                    vt_sb[:, ktl * 128:(ktl + 1) * 128],
                                    idn_sb[:])
                dstv = v_aug[:, gkt, :].rearrange(
                    "p (h e) -> p h e", e=65)[:, :, 0:64]
                nc.vector.tensor_copy(dstv, tp[:].rearrange(
                    "p (h e) -> p h e", e=64))

        # ---------------- collectives ----------------
        a2a_in = [dr.tile([8, 65, TC], BF16, name=f"a2ai{i}", tag=f"a2ai{i}")
                  for i in range(2)]
        a2a_out = [dr.tile([8, 65, TC], BF16, name=f"a2ao{i}", tag=f"a2ao{i}")
                   for i in range(2)]
        at_sb = finp.tile([128, 8, TC], BF16, tag="at")
        den_sb = [finp.tile([8, TC], BF16, name=f"den{i}", tag=f"den{i}")
                  for i in range(2)]
        denf_sb = [finp.tile([8, TC], F32, name=f"denf{i}", tag=f"denf{i}")
                   for i in range(2)]
        rec_sb = [finp.tile([8, TC], F32, name=f"rec{i}", tag=f"rec{i}")
                  for i in range(2)]
        recs_sb = [ropep.tile([8, TC], F32, name=f"recs{i}", tag="rsc")
                   for i in range(2)]
        recb_sb = [finp.tile([8, TC], BF16, name=f"recb{i}", tag=f"recb{i}")
                   for i in range(2)]
        rec_dr = [dr.tile([1, 8 * TC], BF16, name=f"recdr{i}", tag=f"recdr{i}")
                  for i in range(2)]
        ot_sb = finp.tile([128, 8, 512], BF16, tag="ot")

        def emit_a2a(h2):
            for b in range(B):
                src = a_out[h2 * B + b][:].rearrange("p (tc c) -> p tc c",
                                                     tc=4)
                dst = a2a_in[h2][4 * b:4 * b + 4, :, :].rearrange(
                    "j p c -> p j c")
                nc.sync.dma_start(dst, src)
            nc.gpsimd.collective_compute(
                "AllToAll", ALU.bypass, replica_groups=[list(range(N_CORES))],
                ins=[a2a_in[h2].opt()], outs=[a2a_out[h2].opt()])

        def recv_a2a(h2):
            nc.sync.dma_start(den_sb[h2][:], a2a_out[h2][:, 64, :])
            nc.vector.tensor_copy(denf_sb[h2][:], den_sb[h2][:])
            nc.vector.reciprocal_approx_accurate(
                out=rec_sb[h2][:], in_=denf_sb[h2][:], scratch=recs_sb[h2][:])
            nc.vector.tensor_copy(recb_sb[h2][:], rec_sb[h2][:])
            nc.sync.dma_start(
                rec_dr[h2][:].rearrange("o (c t) -> (o c) t", c=8),
                recb_sb[h2][:])
            p0 = 64 * h2
            nc.sync.dma_start(
                at_sb[p0:p0 + 64, :, :],
                a2a_out[h2][:, 0:64, :].rearrange("c p t -> p c t"))
            r_sc = ropep.tile([128, 8, TC], BF16, tag="rbc", bufs=1)
            nc.sync.dma_start(
                r_sc[p0:p0 + 64, :, :].rearrange("p c t -> p (c t)"),
                rec_dr[h2][:].to_broadcast((64, 8 * TC)))
            nc.vector.tensor_mul(at_sb[p0:p0 + 64, :, :],
                                 at_sb[p0:p0 + 64, :, :],
                                 r_sc[p0:p0 + 64, :, :])

        def emit_wo(h2):
            p0 = 64 * h2
            for tt in range(4):
                for mh in range(2):
                    po = ps.tile([128, 512], F32, tag="pp")
                    for c in range(8):
                        nc.tensor.matmul(
                            po[:],
                            at_sb[p0:p0 + 64, c, tt * 128:(tt + 1) * 128],
                            wo_sb[p0:p0 + 64, c, mh * 512:(mh + 1) * 512],
                            start=(c == 0), stop=(c == 7))
                    if h2 == 0:
                        nc.vector.tensor_copy(ot_sb[:, 2 * tt + mh, :], po[:])
                    else:
                        ob = ropep.tile([128, 512], BF16, tag="ob")
                        nc.vector.tensor_add(ob[:], po[:],
                                             ot_sb[:, 2 * tt + mh, :])
                        nc.sync.dma_start(
                            out_d[tt * 128:(tt + 1) * 128,
                                  mh * 512:(mh + 1) * 512], ob[:])

        # ---------------- attention ----------------
        a_out = [None] * 4   # [65, T] numerator^T (+den row 64) per (h2, b)
        e_sb = {}

        def make_e(h2):
            e_sb[h2] = {}
            for b in range(B):
                a_out[h2 * B + b] = ap_.tile([65, T], BF16, name=f"a{h2}{b}",
                                             tag=f"a{b}")
                e_sb[h2][b] = ep.tile([128, ETOT], BF16, name=f"e{h2}{b}",
                                      tag="E")

        def emit_av(h2, b, qc):
            q0 = qc * 512
            kts = [kt for kt in range(KT)
                   if window(kt * 128)[0] < q0 + 512
                   and window(kt * 128)[1] > q0]
            av = ps.tile([65, 512], F32, tag="pp", padded_shape=[128, 512])
            for i, kt in enumerate(kts):
                ws, we = window(kt * 128)
                lo = max(q0, ws)
                hi = min(q0 + 512, we)
                nc.tensor.matmul(
                    av[:, lo - q0:hi - q0],
                    v_aug[:, b * KT + kt, 65 * h2:65 * h2 + 65],
                    e_sb[h2][b][:, EOFF[kt] + lo - ws:EOFF[kt] + hi - ws],
                    start=(i == 0), stop=(i == len(kts) - 1))
            if qc >= 2:
                # late chunks: ACT is idle right after the final exps
                nc.scalar.activation(a_out[h2 * B + b][:, q0:q0 + 512],
                                     av[:], AF.Copy)
            else:
                # mid-stream chunks: keep them off ACT (the pacing engine)
                nc.vector.tensor_copy(a_out[h2 * B + b][:, q0:q0 + 512],
                                      av[:])

        def do_kt(h2, kt):
            k0 = kt * 128
            ws, we = window(k0)
            W = we - ws
            sc = {}
            for b in range(B):
                p0 = 64 * h2
                sc[b] = ps.tile([128, W], F32, name=f"s{h2}{kt}{b}", tag="sc",
                                padded_shape=[128, MAXW])
                for s0 in range(0, W, 512):
                    s1 = min(s0 + 512, W)
                    nc.tensor.matmul(
                        sc[b][:, s0:s1],
                        krope[p0:p0 + 64, b, k0:k0 + 128],
                        qrope[p0:p0 + 64, b, ws + s0:ws + s1],
                        start=True, stop=True,
                        tile_position=(p0, 0))
            eo = EOFF[kt]
            for b in range(B):
                nc.scalar.activation(e_sb[h2][b][:, eo:eo + W], sc[b][:, 0:W],
                                     AF.Exp, scale=SCALE)
            for b in range(B):
                if k0 >= WIN:
                    nc.vector.tensor_mul(e_sb[h2][b][:, eo:eo + 128],
                                         e_sb[h2][b][:, eo:eo + 128],
                                         tl_sb[:])
                if k0 + 128 + WIN <= T:
                    nc.vector.tensor_mul(
                        e_sb[h2][b][:, eo + W - 128:eo + W],
                        e_sb[h2][b][:, eo + W - 128:eo + W], tr_sb[:])
            for b in range(B):
                for qc in range(4):
                    if min(4 * qc + 7, KT - 1) == kt:
                        emit_av(h2, b, qc)

        # software-pipelined emission: fp8 QK chunks stream first (small,
        # critical path); V chunks trail (first needed at kt7's AV); h2=0
        # k-tiles interleave as soon as their q-window is covered
        nc.sync.dma_start(cos_sb[:], cos_d[:])
        nc.sync.dma_start(sin_sb[:], sin_d[:])
        nc.sync.dma_start(wv_sb[:], wv_d[:])
        nc.sync.dma_start(idn_sb[:], idn_d[:])
        nc.sync.dma_start(tl_sb[:], tl_d[:])
        nc.sync.dma_start(tr_sb[:], tr_d[:])
        for tq in range(4):
            for b in range(B):
                do_chunk_qk(b, tq)
        make_e(0)
        for kt in range(0, 6):
            do_kt(0, kt)
        # V projections deferred (first consumer is kt7's AV); x is
        # re-loaded for this pass -- the DMA engines are idle by now
        for tq in range(4):
            for b in range(B):
                do_chunk_v(b, tq)
        nc.sync.dma_start(wo_sb[:], wo_d[:])
        for kt in range(6, KT):
            do_kt(0, kt)
        emit_a2a(0)
        if dbg:
            nc.sync.dma_start(dbg["dbg_q"][:], qrope[:])
            nc.sync.dma_start(dbg["dbg_k"][:], krope[:])
            nc.sync.dma_start(dbg["dbg_vaug"][:], v_aug[:])
            nc.sync.dma_start(dbg["dbg_e0"][:], e_sb[0][0][:])
            nc.sync.dma_start(dbg["dbg_e1"][:], e_sb[0][1][:])
            nc.sync.dma_start(dbg["dbg_a"][:], a_out[0][:])
        make_e(1)
        for kt in range(KT):
            do_kt(1, kt)
        emit_a2a(1)
        # emitted here (not right after collective #1) so the in-order
        # engine queues never make h2=1's work wait on collective #1
        recv_a2a(0)
        emit_wo(0)
        # keep the PE p-state hot through collective #2: independent filler
        # matmuls that run only while PE would otherwise idle (Wo(1) waits
        # on the collective; these have no downstream consumers)
        for _ in range(2):
            fp = ps.tile([128, 512], F32, tag="pp")
            for i in range(32):
                nc.tensor.matmul(fp[:], wo_sb[0:64, 0, 0:128],
                                 wo_sb[0:64, 1, 0:512],
                                 start=(i == 0), stop=(i == 31))
        recv_a2a(1)
        emit_wo(1)


# ---------------------------------------------------------------------------
# Self-contained entry point: kernel(**inputs) -> full output [2, 2048, 1024]
# ---------------------------------------------------------------------------
_CACHE = {}


def _get_nc():
    if "nc" in _CACHE:
        return _CACHE["nc"]
    import concourse.bacc as _bacc
    nc = _bacc.Bacc("TRN2", target_bir_lowering=False, debug=False,
                    num_devices=N_CORES)
    build(nc)
    _CACHE["nc"] = nc
    return nc


def kernel(x, Wq, Wk, Wv, Wo):
    from concourse.bass_utils import run_bass_kernel_spmd
    x, Wq, Wk, Wv, Wo = (np.asarray(a, np.float32) for a in (x, Wq, Wk, Wv, Wo))
    nc = _get_nc()
    in_maps = [host_inputs(x, Wq, Wk, Wv, Wo, c) for c in range(N_CORES)]
    res = run_bass_kernel_spmd(nc, in_maps, core_ids=list(range(N_CORES)))
    return host_assemble(res.results)
